# revision 106
# baseline (speedup 1.0000x reference)
"""Bass/Trainium2 kernel for BailingAttention (GQA prefill, causal, RoPE).

Sharding: tensor-parallel over heads across 8 NeuronCores. Each core computes
2 query heads + its group's shared KV head end-to-end and writes a partial
[T, HID] output (bf16); the host sums the 8 partials (row-parallel
all-reduce) and applies the global dequant scale.

Numerics (rel tolerance 2e-2; this lands ~1e-2):
  - QKV and output projections are 3-term split-fp8 DoubleRow matmuls:
    x ~ hi + res, both e4m3 in SHARED scale units, so hi@hi + hi@res +
    res@hi accumulates in one PSUM group at ~0.1% error and 3/4 the fp32r
    PE cost (DoubleRow = 0.5 cyc/row over a 2x128 contraction).
  - Scores stay fp32r (the exp is error-sensitive).
  - exp outputs (e) are e4m3; PV and the softmax denominator run DoubleRow
    over adjacent key-tile PAIRS (e pair tiles [128,2,512]); v is hi+res.
  - All scales are powers of two, folded into host tables (cos/sin carry
    the QKV dequant, the denominator 'ones' stationary carries the ctx
    scale, the final dequant happens host-side during the partial sum).

Schedule: one software-pipelined emission stream. Per 512-token block b:
pass A (q heads) QKV matmuls interleaved with attention(b-1) leftovers and
out-proj(b-1); RoPE(q); pass B (k/v) interleaved with attention(b) head 0's
non-diagonal work. One attention head is in flight at a time so PSUM fits:
QKV ring 2 banks + score-pair ring 4 + ctx 1 + den 1 = 8.

Engine discipline (each engine's FIFO order gates its consumers):
  ACT  = exps, q1 evict, half the out-proj evicts (all of them in the tail).
  DVE  = psum evicts (q0/k/v), RoPE m1+add, v hi/res quantize, softmax
         recip/c32/ctx_hi, the other out-proj evicts.
  Pool = RoPE t2, causal tri-masks+memsets (shrunk to [128,128]), ctx_re.
  SP   = every DMA (inputs, xsw swaps, v DMA-transpose, outputs-delayed);
         all input DMAs are dependency-free so the SP queue never blocks.
Diagonal score pairs are exact-causal: the moving range shrinks to >=256
columns, exp shrinks with it, masks act on [128,128] triangles only.
Consumes trail their score pairs by 2 (e pairs buffer in epool), and
head 1's non-diagonal consumes slip a full block so its exps hide under
QKV matmuls instead of serializing the tail.
"""

import numpy as np
import ml_dtypes

import concourse.bass as bass
import concourse.mybir as mybir
import concourse.tile as tile
from concourse import bacc, bass_utils
from concourse.bass import ts

F32 = mybir.dt.float32
F32R = mybir.dt.float32r
F8 = mybir.dt.float8e4
BF16 = mybir.dt.bfloat16
AF = mybir.ActivationFunctionType
OP = mybir.AluOpType
DR = mybir.MatmulPerfMode.DoubleRow
NPF8 = ml_dtypes.float8_e4m3
NPBF = ml_dtypes.bfloat16

H, KV, D, HID, T = 16, 4, 128, 2048, 2048
THETA = 10000.0
N_CORES = 8
QH = H // N_CORES            # query heads per core = 2
TB = 512                     # token block (matmul moving N)
NTB = T // TB                # 4
NPAIR = HID // 256           # 8 contraction pairs for QKV
SCALE = float(D) ** -0.5

S_H = 32.0                   # hidden quant scale
S_W = 1024.0                 # Wqkv quant scale
S_V = 4.0                    # on-device v scale (v tiles = 4*v_true)
S_ADJ = S_V / (S_H * S_W)    # v psum -> scaled-v evict factor
ALPHA = 0.125                # ones value: ctx_hi = (S_V/ALPHA)*ctx = 32*ctx
S_WO = 1024.0                # Wo quant scale
S_OUT = (S_V / ALPHA) * S_WO  # host-side dequant of the partial outputs


def _riffle(a, b):
    """Proportionally merge two unit lists into one."""
    if not a or not b:
        return a + b
    out = []
    r = len(a) / len(b)
    bi = 0
    acc = 0.0
    for u in a:
        out.append(u)
        acc += 1.0
        while bi < len(b) and acc >= r:
            out.append(b[bi])
            bi += 1
            acc -= r
    out.extend(b[bi:])
    return out


def _interleave(main, filler):
    """Emit `main` and `filler` unit lists proportionally merged."""
    if not filler or not main:
        for u in main + filler:
            u()
        return
    r = len(main) / len(filler)
    fi = 0
    acc = 0.0
    for u in main:
        u()
        acc += 1.0
        while fi < len(filler) and acc >= r:
            filler[fi]()
            fi += 1
            acc -= r
    while fi < len(filler):
        filler[fi]()
        fi += 1


def _build():
    nc = bacc.Bacc("TRN2", target_bir_lowering=False, debug=False,
                   num_devices=N_CORES)

    h_hi_d = nc.dram_tensor("h_hi", [HID, T], F8, kind="ExternalInput").ap()
    h_re_d = nc.dram_tensor("h_re", [HID, T], F8, kind="ExternalInput").ap()
    # w split by output-column pair: A = n in (0,1) [q heads], B = n in (2,3)
    w_d = {}
    for hv in ("hi", "re"):
        for ab in ("A", "B"):
            w_d[(hv, ab)] = nc.dram_tensor(
                f"w_{hv}{ab}", [128, NPAIR, 2, 2, 128], F8,
                kind="ExternalInput").ap()
    wo_hi_d = nc.dram_tensor("wo_hi", [128, 2, HID], F8,
                             kind="ExternalInput").ap()
    wo_re_d = nc.dram_tensor("wo_re", [128, 2, HID], F8,
                             kind="ExternalInput").ap()
    cos_d = nc.dram_tensor("cosT", [128, T], BF16, kind="ExternalInput").ap()
    sin_d = nc.dram_tensor("sinT", [128, T], BF16, kind="ExternalInput").ap()
    mask_d = nc.dram_tensor("masks", [128, 128], F8, kind="ExternalInput").ap()
    ones_d = nc.dram_tensor("ones", [128, 2, 128], F8, kind="ExternalInput").ap()
    out_d = nc.dram_tensor("out_partial", [T, HID], BF16, kind="ExternalOutput").ap()

    with tile.TileContext(nc) as tc:
        with tc.tile_pool(name="const", bufs=1) as cpool, \
             tc.tile_pool(name="acts", bufs=1) as apool, \
             tc.tile_pool(name="hstream", bufs=2) as hpool, \
             tc.tile_pool(name="rope", bufs=2) as tpool, \
             tc.tile_pool(name="exp", bufs=12) as epool, \
             tc.tile_pool(name="ctmp", bufs=3) as t2pool, \
             tc.tile_pool(name="outsb", bufs=3) as opool, \
             tc.tile_pool(name="qkvps", bufs=2, space="PSUM") as qkv_ps, \
             tc.tile_pool(name="sps", bufs=2, space="PSUM") as spool, \
             tc.tile_pool(name="cps", bufs=1, space="PSUM") as cpsp, \
             tc.tile_pool(name="dps", bufs=1, space="PSUM") as dpsp:

            w_sb = {k: cpool.tile([128, NPAIR, 2, 2, 128], F8,
                                  name=f"w_{k[0]}{k[1]}", tag=f"w_{k[0]}{k[1]}")
                    for k in w_d}
            wo_hi = cpool.tile([128, 2, HID], F8)
            wo_re = cpool.tile([128, 2, HID], F8)
            cos_sb = cpool.tile([128, T], BF16)
            sin_sb = cpool.tile([128, T], BF16)
            mask_sb = cpool.tile([128, 128], F8)
            ones_sb = cpool.tile([128, 2, 128], F8)

            qrT = [[apool.tile([128, TB], F32R, name=f"q{i}b{b}", tag=f"q{i}b{b}")
                    for b in range(NTB)] for i in range(QH)]
            krT = [apool.tile([128, TB], F32R, name=f"kb{b}", tag=f"kb{b}")
                   for b in range(NTB)]
            vbf = [apool.tile([128, 4, 128], BF16, name=f"vbf{b}", tag=f"vbf{b}")
                   for b in range(NTB)]
            v_hi = [apool.tile([128, 2, 2, 128], F8, name=f"vhb{b}", tag=f"vhb{b}")
                    for b in range(NTB)]
            v_re = [apool.tile([128, 2, 2, 128], F8, name=f"vrb{b}", tag=f"vrb{b}")
                    for b in range(NTB)]
            ctx_hi = [apool.tile([128, 2, TB], F8, name=f"chb{b}", tag=f"chb{b}")
                      for b in range(NTB)]
            ctx_re = [apool.tile([128, 2, TB], F8, name=f"crb{b}", tag=f"crb{b}")
                      for b in range(NTB)]

            h_hi_v = h_hi_d.rearrange("(j i p) t -> p j i t", i=2, p=128)
            h_re_v = h_re_d.rearrange("(j i p) t -> p j i t", i=2, p=128)

            h_tiles = {}
            qkv_state = {}
            att_state = {}
            rr = {"osb": 0}

            def u_dma_h(b, hv):
                """Load one h stream (hi or re) for block b as two half-slabs
                on the SP queue."""
                def run():
                    src = h_hi_v if hv == "hi" else h_re_v
                    for ab, j0 in (("A", 0), ("B", 4)):
                        t = hpool.tile([128, 4, 2, TB], F8, tag=f"h{hv}{ab}")
                        h_tiles[(b, hv, ab)] = t
                        nc.sync.dma_start(t[:], src[:, j0:j0 + 4, :, ts(b, TB)])
                return run

            def u_dma_w(hv, ab):
                def run():
                    nc.sync.dma_start(w_sb[(hv, ab)][:], w_d[(hv, ab)])
                return run

            def u_dma_tables(b):
                def run():
                    nc.sync.dma_start(cos_sb[:, ts(b, TB)], cos_d[:, ts(b, TB)])
                    nc.sync.dma_start(sin_sb[:, ts(b, TB)], sin_d[:, ts(b, TB)])
                    if b == 0:
                        nc.sync.dma_start(ones_sb[:], ones_d)
                        nc.sync.dma_start(mask_sb[:], mask_d)
                return run

            def u_dma_const1():
                def run():
                    nc.sync.dma_start(wo_hi[:], wo_hi_d)
                    nc.sync.dma_start(wo_re[:], wo_re_d)
                return run

            # ---------------- QKV stream ------------------------------------
            def u_qkv_mm(b, n, stream, jh):
                """One unit = 4 DoubleRow matmuls (j = jh*4 .. jh*4+3)."""
                def run():
                    ab = "A" if jh == 0 else "B"
                    wab = "A" if n < 2 else "B"
                    if stream == 0 and jh == 0:
                        qkv_state[(b, n)] = qkv_ps.tile(
                            [128, TB], F32, name=f"qkv{n}", tag="qkv")
                    ps = qkv_state[(b, n)]
                    wv, hv = [("hi", "hi"), ("hi", "re"), ("re", "hi")][stream]
                    wt = w_sb[(wv, wab)]
                    ht = h_tiles[(b, hv, ab)]
                    for jj in range(4):
                        j = jh * 4 + jj
                        nc.tensor.matmul(
                            ps[:], wt[:, j, :, n % 2, :], ht[:, jj],
                            perf_mode=DR,
                            start=(stream == 0 and j == 0),
                            stop=(stream == 2 and j == NPAIR - 1))
                return run

            def u_evict_rope(b, n):
                """Evict qkv psum n (q0/q1/k) and run its RoPE chain."""
                def run():
                    ps = qkv_state.pop((b, n))
                    x_sb = tpool.tile([128, TB], BF16, tag=f"x{n}")
                    if n == 1:
                        nc.scalar.copy(x_sb[:], ps[:])
                    else:
                        nc.vector.tensor_copy(x_sb[:], ps[:])
                    dst = qrT[n][b] if n < QH else krT[b]
                    xsw = tpool.tile([128, TB], BF16, tag=f"xsw{n}")
                    nc.sync.dma_start(xsw[0:64, :], x_sb[64:128, :])
                    nc.sync.dma_start(xsw[64:128, :], x_sb[0:64, :])
                    t2 = tpool.tile([128, TB], BF16, tag=f"t2{n}")
                    m1 = tpool.tile([128, TB], BF16, tag=f"m1{n}")
                    nc.vector.tensor_tensor(out=m1[:], in0=x_sb[:],
                                            in1=cos_sb[:, ts(b, TB)], op=OP.mult)
                    nc.gpsimd.tensor_tensor(out=t2[:], in0=xsw[:],
                                            in1=sin_sb[:, ts(b, TB)], op=OP.mult)
                    nc.vector.tensor_tensor(out=dst[:], in0=m1[:], in1=t2[:],
                                            op=OP.add)
                return run

            def u_evict_v(b):
                def run():
                    ps = qkv_state.pop((b, 3))
                    x_sb = tpool.tile([128, TB], BF16, tag="xv")
                    nc.vector.tensor_scalar_mul(x_sb[:], ps[:], S_ADJ)
                    qkv_state[("vT", b)] = x_sb
                return run

            def u_vtrans(b, jj):
                def run():
                    vT_sb = qkv_state[("vT", b)]
                    nc.sync.dma_start_transpose(vbf[b][:, jj, :],
                                                  vT_sb[:, ts(jj, 128)])
                return run

            def u_vquant(b, jj):
                def run():
                    p, s = jj // 2, jj % 2
                    nc.vector.tensor_copy(v_hi[b][:, p, s, :], vbf[b][:, jj, :])
                    nc.vector.tensor_tensor(out=v_re[b][:, p, s, :],
                                            in0=vbf[b][:, jj, :],
                                            in1=v_hi[b][:, p, s, :],
                                            op=OP.subtract)
                return run

            # ---------------- attention stream ------------------------------
            def qlo_of(bb, p):
                """Exact-causal moving-range start for pair p of block bb
                (clamped so fp32r keeps >=256 moving columns)."""
                k0 = 2 * p - 4 * bb          # first local key tile of the pair
                if k0 < 0:
                    return 0
                return min(128 * k0, TB - 256)

            def u_score_pair(bb, qh, p):
                """Two fp32r score matmuls + one paired exp (+ diag masks)."""
                def run():
                    st = att_state.setdefault((bb, qh), {})
                    s_pair = spool.tile([128, 2, TB], F32, name="s_pair",
                                        tag="sps")
                    e_pair = epool.tile([128, 2, TB], F8, name="e_pair",
                                        tag="exp")
                    st[("e", p)] = e_pair
                    q0 = qlo_of(bb, p)
                    for s in range(2):
                        kt = 2 * p + s
                        nc.tensor.matmul(s_pair[:, s, q0:TB],
                                         krT[kt // 4][:, ts(kt % 4, 128)],
                                         qrT[qh][bb][:, q0:TB],
                                         start=True, stop=True)
                    nc.scalar.activation(e_pair[:, :, q0:TB],
                                         s_pair[:, :, q0:TB], AF.Exp,
                                         scale=SCALE)
                    if 2 * p + 1 >= 4 * bb:   # diagonal pair: causal masks
                        for s in range(2):
                            kt_l = 2 * p + s - 4 * bb
                            c0 = 128 * kt_l
                            nc.gpsimd.tensor_tensor(
                                out=e_pair[:, s, c0:c0 + 128],
                                in0=e_pair[:, s, c0:c0 + 128],
                                in1=mask_sb[:], op=OP.mult)
                            if s == 1 and c0 > q0:
                                nc.gpsimd.memset(e_pair[:, 1, q0:c0], 0.0)
                return run

            def u_consume(bb, qh, p, npair):
                def run():
                    st = att_state[(bb, qh)]
                    if p == 0:
                        st["ctx"] = cpsp.tile([128, TB], F32, name="ctx_ps")
                        st["den"] = dpsp.tile([128, TB], F32, name="den_ps")
                    e_pair = st.pop(("e", p))
                    first = (p == 0)
                    last = (p == npair - 1)
                    q0 = qlo_of(bb, p)
                    vb, vp = p // 2, p % 2
                    nc.tensor.matmul(st["ctx"][:, q0:TB], v_hi[vb][:, vp, :, :],
                                     e_pair[:, :, q0:TB], perf_mode=DR,
                                     start=first, stop=False)
                    nc.tensor.matmul(st["ctx"][:, q0:TB], v_re[vb][:, vp, :, :],
                                     e_pair[:, :, q0:TB], perf_mode=DR,
                                     start=False, stop=last)
                    nc.tensor.matmul(st["den"][:, q0:TB], ones_sb[:],
                                     e_pair[:, :, q0:TB], perf_mode=DR,
                                     start=first, stop=last)
                return run

            def u_ctx1(bb, qh):
                def run():
                    st = att_state[(bb, qh)]
                    recip = t2pool.tile([128, TB], F32, tag="recip")
                    c32 = t2pool.tile([128, TB], F32, tag="c32")
                    nc.vector.reciprocal(recip[:], st["den"][:])
                    nc.vector.tensor_tensor(out=c32[:], in0=st["ctx"][:],
                                            in1=recip[:], op=OP.mult)
                    st["c32"] = c32
                return run

            def u_ctx2(bb, qh):
                def run():
                    st = att_state.pop((bb, qh))
                    c32 = st["c32"]
                    nc.vector.tensor_copy(ctx_hi[bb][:, qh, :], c32[:])
                    nc.gpsimd.tensor_tensor(out=ctx_re[bb][:, qh, :],
                                            in0=c32[:],
                                            in1=ctx_hi[bb][:, qh, :],
                                            op=OP.subtract)
                return run

            def att_units(bb, qh, part):
                """nd: scores+consumes for non-diagonal pairs; nd_s/nd_c:
                scores-only / consumes-only variants (e pairs buffered in
                epool between them); d: diagonal pairs + softmax chain."""
                npair = 2 * (bb + 1)
                if part in ("nd", "nd_s", "nd_c"):
                    pairs = range(0, 2 * bb)
                else:
                    pairs = range(2 * bb, npair)
                units = []
                pend = []
                for p in pairs:
                    if part != "nd_c":
                        units.append(u_score_pair(bb, qh, p))
                    if part != "nd_s":
                        if part in ("nd", "d"):
                            pend.append(u_consume(bb, qh, p, npair))
                            if len(pend) > 2:
                                units.append(pend.pop(0))
                        else:
                            units.append(u_consume(bb, qh, p, npair))
                units.extend(pend)
                if part == "d":
                    units.append(u_ctx1(bb, qh))
                    units.append(u_ctx2(bb, qh))
                return units

            # ---------------- output-projection stream ----------------------
            def out_units(bb):
                units = []
                st = {}

                def u_alloc(tt, st=st):
                    def run():
                        st[tt] = opool.tile([128, 2, 2, TB], BF16, name="o_sb")
                    return run

                def u_proj(tt, half, bb=bb, st=st):
                    def run():
                        ps = spool.tile([128, 2, TB], F32, name="ps_o",
                                        tag="sps")
                        ch = ctx_hi[bb][:, :, ts(tt % 4, 128)]
                        cr = ctx_re[bb][:, :, ts(tt % 4, 128)]
                        for s in range(2):
                            n = 2 * half + s
                            nc.tensor.matmul(ps[:, s, :], ch,
                                             wo_hi[:, :, ts(n, 512)],
                                             perf_mode=DR, start=True, stop=False)
                            nc.tensor.matmul(ps[:, s, :], ch,
                                             wo_re[:, :, ts(n, 512)],
                                             perf_mode=DR, start=False, stop=False)
                            nc.tensor.matmul(ps[:, s, :], cr,
                                             wo_hi[:, :, ts(n, 512)],
                                             perf_mode=DR, start=False, stop=True)
                        rr["osb"] += 1
                        if bb != NTB - 1 and rr["osb"] % 2:
                            nc.vector.tensor_copy(st[tt][:, half, :, :], ps[:])
                        else:
                            nc.scalar.copy(st[tt][:, half, :, :], ps[:])
                    return run

                def u_odma(tt, half, st=st):
                    def run():
                        o = st[tt] if half == 0 else st.pop(tt)
                        nc.sync.dma_start(
                            out_d[ts(tt, 128), ts(half, 1024)], o[:, half])
                    return run

                # emit the DMA for tile tt after the next tile's first proj
                # so the SP queue never head-of-line blocks on the evict.
                pend = []
                for tt in range(4 * bb, 4 * bb + 4):
                    units.append(u_alloc(tt))
                    units.append(u_proj(tt, 0))
                    if pend:
                        units.append(pend.pop(0))
                    units.append(u_proj(tt, 1))
                    pend.append(u_odma(tt, 0))
                    pend.append(u_odma(tt, 1))
                units += pend
                return units

            # ---------------- merged emission --------------------------------
            def qkv_pass(b, ns):
                units = []
                for stream in range(3):
                    for n in ns:
                        for jh in range(2):
                            units.append(u_qkv_mm(b, n, stream, jh))
                return units

            for b in range(NTB):
                # ---- pass A (q heads) ----
                mainA = []
                if b == 0:
                    mainA.append(u_dma_w("hi", "A"))
                    mainA.append(u_dma_h(0, "hi"))
                    mainA.append(u_dma_h(0, "re"))
                    mainA.append(u_dma_w("re", "A"))
                mainA += qkv_pass(b, (0, 1))
                if b == 0:
                    mainA.insert(7, u_dma_w("hi", "B"))
                    mainA.insert(8, u_dma_w("re", "B"))
                else:
                    mainA.insert(6, u_dma_h(b + 1, "hi") if b + 1 < NTB
                                 else (lambda: None))
                mainA.append(u_dma_tables(b))
                if b + 1 < NTB:
                    if b == 0:
                        mainA.append(u_dma_h(b + 1, "hi"))
                    mainA.append(u_dma_h(b + 1, "re"))
                fillerA = []
                if b > 0:
                    fillerA += [u_vtrans(b - 1, jj) for jj in range(4)]
                    fillerA += [u_vquant(b - 1, jj) for jj in range(4)]
                    fillerA += att_units(b - 1, 0, "d")
                    fillerA += att_units(b - 1, 1, "nd_c")
                    fillerA += att_units(b - 1, 1, "d")
                _interleave(mainA, fillerA)
                # ---- RoPE for q heads ----
                u_evict_rope(b, 0)()
                u_evict_rope(b, 1)()
                # ---- pass B (k, v) ----
                mainB = qkv_pass(b, (2, 3))
                if b == 0:
                    mainB.append(u_dma_const1())
                att_b = att_units(b, 0, "nd") + att_units(b, 1, "nd_s")
                fillerB = att_b + (out_units(b - 1) if b > 0 else [])
                _interleave(mainB, fillerB)
                u_evict_rope(b, 2)()
                u_evict_v(b)()

            # ---- tail: attention(last) + out(last) ----
            bl = NTB - 1
            tail = [u_vtrans(bl, jj) for jj in range(4)]
            tail += [u_vquant(bl, jj) for jj in range(4)]
            tail += att_units(bl, 0, "d")
            tail += att_units(bl, 1, "nd_c")
            tail += att_units(bl, 1, "d")
            tail += out_units(bl)
            for u in tail:
                u()

    nc.compile()
    return nc


_NC_CACHE = None


def _get_nc():
    global _NC_CACHE
    if _NC_CACHE is None:
        _NC_CACHE = _build()
    return _NC_CACHE


def _f8(x):
    return np.asarray(x, np.float32).astype(NPF8)


def _host_tables(position_ids: np.ndarray):
    pos = np.asarray(position_ids, np.float32)
    inv_freq = (1.0 / (THETA ** (np.arange(0, D, 2, dtype=np.float32) / D)))
    ang = pos[:, None] * inv_freq[None, :]          # [T, 64] f32
    dq = np.float32(1.0 / (S_H * S_W))              # QKV dequant folded in
    cos = (np.cos(ang).T * dq).astype(np.float32)   # [64, T]
    sin = (np.sin(ang).T * dq).astype(np.float32)
    cosT = np.concatenate([cos, cos], axis=0).astype(NPBF)   # [128, T]
    sinT = np.concatenate([-sin, sin], axis=0).astype(NPBF)
    return cosT, sinT


def _host_mask():
    r = np.arange(128)[:, None]
    c = np.arange(128)[None, :]
    return _f8((c - r >= 0).astype(np.float32))     # [128, 128] triangle


def _pack_w(w_local_scaled: np.ndarray):
    """[2048, 512] (already scaled) -> hi/res packed [128, 8, 2, 4, 128]."""
    hi = _f8(w_local_scaled)
    re = _f8(w_local_scaled - hi.astype(np.float32))

    def pack(a):
        # row r = j*256 + i*128 + p ; col = n*128 + c
        return np.ascontiguousarray(
            a.reshape(NPAIR, 2, 128, 4, 128).transpose(2, 0, 1, 3, 4))
    return pack(hi), pack(re)


def kernel(hidden_states, position_ids, Wqkv, Wo):
    hidden_states = np.asarray(hidden_states, np.float32)
    Wqkv = np.asarray(Wqkv, np.float32)
    Wo = np.asarray(Wo, np.float32)

    nc = _get_nc()

    hT_s = hidden_states.T * np.float32(S_H)
    h_hi = _f8(hT_s)
    h_re = _f8(hT_s - h_hi.astype(np.float32))
    cosT, sinT = _host_tables(position_ids)
    mask = _host_mask()
    ones = np.full((128, 2, 128), ALPHA, np.float32).astype(NPF8)

    wq = Wqkv[:, : H * D]
    wk = Wqkv[:, H * D: (H + KV) * D]
    wv = Wqkv[:, (H + KV) * D:]

    in_maps = []
    for c in range(N_CORES):
        kvh = (c * QH) // (H // KV)
        w_local = np.concatenate(
            [wq[:, (c * QH) * D: (c * QH + 1) * D],
             wq[:, (c * QH + 1) * D: (c * QH + 2) * D],
             wk[:, kvh * D: (kvh + 1) * D],
             wv[:, kvh * D: (kvh + 1) * D]], axis=1) * np.float32(S_W)
        w_hi, w_re = _pack_w(w_local)   # [128, 8, 2, 4, 128]
        w_split = {
            "w_hiA": np.ascontiguousarray(w_hi[:, :, :, 0:2, :]),
            "w_hiB": np.ascontiguousarray(w_hi[:, :, :, 2:4, :]),
            "w_reA": np.ascontiguousarray(w_re[:, :, :, 0:2, :]),
            "w_reB": np.ascontiguousarray(w_re[:, :, :, 2:4, :]),
        }
        wo_local = Wo[c * QH * D: (c + 1) * QH * D, :] * np.float32(S_WO)
        wo_hi = _f8(wo_local)
        wo_re = _f8(wo_local - wo_hi.astype(np.float32))
        # [2*128, HID] -> [128, 2, HID] (slot = head)
        wo_hi = np.ascontiguousarray(wo_hi.reshape(2, 128, HID).transpose(1, 0, 2))
        wo_re = np.ascontiguousarray(wo_re.reshape(2, 128, HID).transpose(1, 0, 2))
        in_maps.append({
            "h_hi": h_hi, "h_re": h_re,
            **w_split,
            "wo_hi": wo_hi, "wo_re": wo_re,
            "cosT": cosT, "sinT": sinT, "masks": mask,
            "ones": ones,
        })

    res = bass_utils.run_bass_kernel_spmd(nc, in_maps,
                                          core_ids=list(range(N_CORES)))
    parts = np.stack([res.results[c]["out_partial"].astype(np.float32)
                      for c in range(N_CORES)], 0)
    return parts.sum(axis=0, dtype=np.float32) * np.float32(1.0 / S_OUT)
